# revision 5
# baseline (speedup 1.0000x reference)
"""Trainium2 Bass kernel for nn_BertSelfOutput_79448305042105.

Computes, for two streams (hs1,W1,b1,g1,be1) and (hs2,W2,b2,g2,be2):
    h   = quant(hs) @ quant(W).T + b          (symmetric 8-bit quant-dequant)
    h   = dropout(h, p=0.1, jax key 42)
    out = layernorm(h + input_tensor) * g + be

Sharding: data-parallel over batch, 2 batches (2048 tokens) per core, 8 cores.

Numerics: quantized values are integers in [-127, 127], exactly representable
in bf16; integer products accumulate exactly in fp32 PSUM (max |sum| < 2^24),
so the matmul runs at full bf16 speed with zero quantization-grid error.
f32->int32 conversion on the vector/scalar engines is round-to-nearest-even,
matching jnp.round. Dropout masks depend only on the fixed PRNG key, so they
are precomputed on host (uint8) and applied on-device.
"""

import sys

if "/opt/trn_rl_repo" not in sys.path:
    sys.path.insert(0, "/opt/trn_rl_repo")

import numpy as np

import concourse.bass as bass
import concourse.tile as tile
from concourse import mybir
from concourse.bass_utils import run_bass_kernel_spmd

# ---------------------------------------------------------------- constants
B, S, H = 16, 1024, 1024
N_CORES = 8
B_PER_CORE = B // N_CORES            # 2
M = B_PER_CORE * S                   # 2048 tokens per core
M_TILES = M // 128                   # 16
H_TILES = H // 128                   # 8
O_CHUNK = 512                        # psum bank free dim
O_CHUNKS = H // O_CHUNK              # 2
GRP = 4                              # m-tiles per stats group
N_GRPS = M_TILES // GRP

CLIP = 2.5
WBITS = IBITS = 8
QN = float(2 ** (IBITS - 1) - 1)     # 127
SCALE = CLIP / QN                    # quant step
INV_SCALE = np.float32(QN / CLIP)
KEEP = 0.9
SP = np.float32((CLIP / QN) * (CLIP / QN) / KEEP)   # psum*SP = dequant/keep
EPS = 1e-12

F32 = mybir.dt.float32
BF16 = mybir.dt.bfloat16
I32 = mybir.dt.int32
U8 = mybir.dt.uint8
Alu = mybir.AluOpType
Act = mybir.ActivationFunctionType

_MASK_CACHE = {}
_PROG_CACHE = {}


# ------------------------------------------------------- walrus wait limit
def _split_sync_waits(nc, max_waits=1):
    """This toolchain's walrus rejects instructions with >1 sync wait.
    Move extra waits onto same-engine NoOps placed just before."""
    ctr = 0
    for f in nc.m.functions:
        for bb in f.blocks:
            out, changed = [], False
            for ins in bb.instructions:
                si = getattr(ins, "sync_info", None)
                waits = list(si.on_wait) if (si is not None and si.on_wait) else []
                if len(waits) > max_waits:
                    changed = True
                    for w in waits[:-max_waits]:
                        ctr += 1
                        out.append(
                            mybir.InstNoOp(
                                name=f"WSPLIT-{ctr}",
                                sync_info=mybir.SyncInfo(on_wait=[w], on_update=[]),
                                bass_nofuse=True,
                                engine=ins.engine,
                            )
                        )
                    ins.sync_info = mybir.SyncInfo(
                        on_wait=waits[-max_waits:],
                        on_update=list(si.on_update) if si.on_update else [],
                    )
                out.append(ins)
            if changed:
                bb.instructions = out


# ------------------------------------------------------------ host helpers
def _dropout_masks():
    """uint8 keep-masks for the two dropout calls (jax key 42), host-cached."""
    if "m" not in _MASK_CACHE:
        import jax

        cpu = jax.devices("cpu")[0]
        with jax.default_device(cpu):
            dkey = jax.random.key(42)
            k1, k2 = jax.random.split(dkey)
            m1 = np.asarray(
                jax.random.bernoulli(k1, KEEP, (B, S, H))
            ).astype(np.uint8)
            m2 = np.asarray(
                jax.random.bernoulli(k2, KEEP, (B, S, H))
            ).astype(np.uint8)
        _MASK_CACHE["m"] = (
            np.ascontiguousarray(m1.reshape(N_CORES, M, H)),
            np.ascontiguousarray(m2.reshape(N_CORES, M, H)),
        )
    return _MASK_CACHE["m"]


# ----------------------------------------------------------- program build
def _build_program(use_b, use_gbe):
    """use_b/use_gbe: per-stream flags for nonzero bias / nontrivial g,be."""
    nc = bass.Bass()

    hs1t = nc.declare_dram_parameter("hs1t", [H, M], F32, isOutput=False)
    hs2t = nc.declare_dram_parameter("hs2t", [H, M], F32, isOutput=False)
    inp = nc.declare_dram_parameter("inp", [M, H], F32, isOutput=False)
    m1 = nc.declare_dram_parameter("m1", [M, H], U8, isOutput=False)
    m2 = nc.declare_dram_parameter("m2", [M, H], U8, isOutput=False)
    w1t = nc.declare_dram_parameter("w1t", [H, H], F32, isOutput=False)
    w2t = nc.declare_dram_parameter("w2t", [H, H], F32, isOutput=False)
    extras = {}
    for s in (0, 1):
        if use_b[s]:
            # pre-divided by SCALE^2 on host: accumulated into integer psum
            extras[f"bs{s}"] = nc.declare_dram_parameter(
                f"bs{s}", [1, H], F32, isOutput=False
            )
        if use_gbe[s]:
            extras[f"g{s}"] = nc.declare_dram_parameter(
                f"g{s}", [1, H], F32, isOutput=False
            )
            extras[f"be{s}"] = nc.declare_dram_parameter(
                f"be{s}", [1, H], F32, isOutput=False
            )
    out1 = nc.declare_dram_parameter("out1", [M, H], F32, isOutput=True)
    out2 = nc.declare_dram_parameter("out2", [M, H], F32, isOutput=True)

    hst = (hs1t, hs2t)
    msk = (m1, m2)
    wt = (w1t, w2t)
    outs = (out1, out2)

    with tile.TileContext(nc) as tc:
        import contextlib

        with contextlib.ExitStack() as ctx:
            const_p = ctx.enter_context(tc.tile_pool(name="const", bufs=1))
            wq_p = ctx.enter_context(tc.tile_pool(name="wq", bufs=1))
            xq_p = ctx.enter_context(tc.tile_pool(name="xq", bufs=1))
            stage_f = ctx.enter_context(tc.tile_pool(name="stage_f", bufs=2))
            stage_i = ctx.enter_context(tc.tile_pool(name="stage_i", bufs=2))
            inp_p = ctx.enter_context(tc.tile_pool(name="inp", bufs=5))
            msk_p = ctx.enter_context(tc.tile_pool(name="msk", bufs=4))
            t2_p = ctx.enter_context(tc.tile_pool(name="t2", bufs=3))
            t3_p = ctx.enter_context(tc.tile_pool(name="t3", bufs=8))
            out_p = ctx.enter_context(tc.tile_pool(name="outp", bufs=3))
            st_p = ctx.enter_context(tc.tile_pool(name="stats", bufs=4))
            psum_p = ctx.enter_context(
                tc.tile_pool(name="psum", bufs=8, space="PSUM")
            )

            eps_t = const_p.tile([128, 1], F32)
            nc.vector.memset(eps_t, EPS)

            # one-partition constants for optional bias / gamma / beta
            bias_rows = {}
            gbe_rows = {}
            ones_col = None
            for s in (0, 1):
                if use_b[s]:
                    bs = const_p.tile([1, H], F32, tag=f"bs{s}")
                    nc.sync.dma_start(out=bs, in_=extras[f"bs{s}"][:, :])
                    bias_rows[s] = bs
                    if ones_col is None:
                        ones_col = const_p.tile([1, 128], F32)
                        nc.vector.memset(ones_col, 1.0)
                if use_gbe[s]:
                    gb = const_p.tile([128, H], F32, tag=f"gb{s}")
                    beb = const_p.tile([128, H], F32, tag=f"beb{s}")
                    src_g = extras[f"g{s}"]
                    src_be = extras[f"be{s}"]
                    # partition-broadcast DMA: replicate the [1, H] row
                    nc.sync.dma_start(
                        out=gb,
                        in_=bass.AP(
                            tensor=src_g.tensor if hasattr(src_g, "tensor") else src_g,
                            offset=0,
                            ap=[[0, 128], [1, H]],
                        ),
                    )
                    nc.sync.dma_start(
                        out=beb,
                        in_=bass.AP(
                            tensor=src_be.tensor if hasattr(src_be, "tensor") else src_be,
                            offset=0,
                            ap=[[0, 128], [1, H]],
                        ),
                    )
                    gbe_rows[s] = (gb, beb)

            # ---- quantize weights + activations, stream-major so that
            # stream 0's matmuls can start while stream 1 still loads.
            # W: DVE round->i32, DVE clip->bf16 (prologue, DVE idle).
            # X: ACT round->i32, GPSIMD clip->bf16 (keeps DVE free for the
            #    steady-state psum/residual chains).
            wq, xq = [], []
            for s in (0, 1):
                wqs = wq_p.tile([128, H_TILES, H], BF16, tag=f"wq{s}")
                for h in range(H_TILES):
                    for col in range(2):
                        fs = stage_f.tile([128, O_CHUNK], F32, tag="stage_f")
                        nc.sync.dma_start(
                            out=fs,
                            in_=wt[s][
                                h * 128 : (h + 1) * 128,
                                col * O_CHUNK : (col + 1) * O_CHUNK,
                            ],
                        )
                        ii = stage_i.tile([128, O_CHUNK], I32, tag="stage_i")
                        nc.vector.tensor_scalar(
                            ii, fs, float(INV_SCALE), None, Alu.mult
                        )
                        nc.vector.tensor_scalar(
                            wqs[:, h, col * O_CHUNK : (col + 1) * O_CHUNK],
                            ii,
                            -127,
                            127,
                            Alu.max,
                            Alu.min,
                        )
                wq.append(wqs)
                xqs = xq_p.tile([128, H_TILES, M], BF16, tag=f"xq{s}")
                for h in range(H_TILES):
                    for col in range(2):
                        fs = stage_f.tile([128, M // 2], F32, tag="stage_f")
                        nc.sync.dma_start(
                            out=fs,
                            in_=hst[s][
                                h * 128 : (h + 1) * 128,
                                col * (M // 2) : (col + 1) * (M // 2),
                            ],
                        )
                        ii = stage_i.tile([128, M // 2], I32, tag="stage_i")
                        nc.scalar.activation(
                            ii, fs, Act.Copy, bias=0.0, scale=float(INV_SCALE)
                        )
                        nc.gpsimd.tensor_scalar(
                            xqs[:, h, col * (M // 2) : (col + 1) * (M // 2)],
                            ii,
                            -127,
                            127,
                            Alu.max,
                            Alu.min,
                        )
                xq.append(xqs)

            # ---- main loop
            inv_h = 1.0 / float(H)
            for grp in range(N_GRPS):
                mts = [grp * GRP + i for i in range(GRP)]
                inp_tiles = []
                for mt in mts:
                    it = inp_p.tile([128, H], F32, tag="inp")
                    nc.sync.dma_start(
                        out=it, in_=inp[mt * 128 : (mt + 1) * 128, :]
                    )
                    inp_tiles.append(it)

                for s in (0, 1):
                    gsum = st_p.tile([128, GRP], F32, tag="gsum")
                    gssq = st_p.tile([128, GRP], F32, tag="gssq")
                    t3s = []
                    for i, mt in enumerate(mts):
                        mk = msk_p.tile([128, H], U8, tag="msk")
                        nc.sync.dma_start(
                            out=mk, in_=msk[s][mt * 128 : (mt + 1) * 128, :]
                        )
                        # matmul: psum[oc] = sum_h xq[:,h,mt]^T @ wq[:,h,oc]
                        # h-major: both o-chunks of one h share the
                        # stationary operand back-to-back.
                        pss = [
                            psum_p.tile(
                                [128, O_CHUNK], F32, tag="ps", name=f"ps{_oc}"
                            )
                            for _oc in range(O_CHUNKS)
                        ]
                        for h in range(H_TILES):
                            for oc in range(O_CHUNKS):
                                last = h == H_TILES - 1 and not use_b[s]
                                nc.tensor.matmul(
                                    pss[oc],
                                    xq[s][:, h, mt * 128 : (mt + 1) * 128],
                                    wq[s][:, h, oc * O_CHUNK : (oc + 1) * O_CHUNK],
                                    start=(h == 0),
                                    stop=last,
                                )
                        if use_b[s]:
                            for oc in range(O_CHUNKS):
                                nc.tensor.matmul(
                                    pss[oc],
                                    ones_col,
                                    bias_rows[s][
                                        :, oc * O_CHUNK : (oc + 1) * O_CHUNK
                                    ],
                                    start=False,
                                    stop=True,
                                )
                        # dropout+dequant: t2 = (psum * SP) * mask
                        t2 = t2_p.tile([128, H], F32, tag="t2")
                        for oc in range(O_CHUNKS):
                            nc.vector.scalar_tensor_tensor(
                                out=t2[:, oc * O_CHUNK : (oc + 1) * O_CHUNK],
                                in0=pss[oc],
                                scalar=float(SP),
                                in1=mk[:, oc * O_CHUNK : (oc + 1) * O_CHUNK],
                                op0=Alu.mult,
                                op1=Alu.mult,
                            )
                        # residual + row-sum
                        t3 = t3_p.tile([128, H], F32, tag="t3")
                        nc.vector.scalar_tensor_tensor(
                            out=t3,
                            in0=t2,
                            scalar=0.0,
                            in1=inp_tiles[i],
                            op0=Alu.add,
                            op1=Alu.add,
                            accum_out=gsum[:, i : i + 1],
                        )
                        t3s.append(t3)
                        # sum of squares on scalar engine (reuse dead t2)
                        nc.scalar.activation(
                            t2, t3, Act.Square, accum_out=gssq[:, i : i + 1]
                        )

                    # ---- group stats: mu, var, rstd (one Newton step)
                    mu = st_p.tile([128, GRP], F32, tag="mu")
                    nc.vector.tensor_scalar(mu, gsum, inv_h, None, Alu.mult)
                    musq = st_p.tile([128, GRP], F32, tag="musq")
                    nc.vector.tensor_tensor(
                        out=musq, in0=mu, in1=mu, op=Alu.mult
                    )
                    var = st_p.tile([128, GRP], F32, tag="var")
                    nc.vector.scalar_tensor_tensor(
                        out=var,
                        in0=gssq,
                        scalar=inv_h,
                        in1=musq,
                        op0=Alu.mult,
                        op1=Alu.subtract,
                    )
                    std = st_p.tile([128, GRP], F32, tag="std")
                    nc.scalar.activation(
                        std, var, Act.Sqrt, bias=eps_t, scale=1.0
                    )
                    r0 = st_p.tile([128, GRP], F32, tag="r0")
                    nc.vector.reciprocal(r0, std)
                    # Newton: rstd = r0 * (1.5 - 0.5 * vpe * r0^2)
                    vpe = st_p.tile([128, GRP], F32, tag="vpe")
                    nc.vector.tensor_scalar(vpe, var, EPS, None, Alu.add)
                    r2 = st_p.tile([128, GRP], F32, tag="r2")
                    nc.vector.tensor_tensor(out=r2, in0=r0, in1=r0, op=Alu.mult)
                    f = st_p.tile([128, GRP], F32, tag="f")
                    nc.vector.tensor_tensor(out=f, in0=r2, in1=vpe, op=Alu.mult)
                    g = st_p.tile([128, GRP], F32, tag="g")
                    nc.vector.tensor_scalar(g, f, -0.5, 1.5, Alu.mult, Alu.add)
                    rstd = st_p.tile([128, GRP], F32, tag="rstd")
                    nc.vector.tensor_tensor(out=rstd, in0=r0, in1=g, op=Alu.mult)
                    nmr = st_p.tile([128, GRP], F32, tag="nmr")
                    nc.vector.scalar_tensor_tensor(
                        out=nmr, in0=mu, scalar=-1.0, in1=rstd,
                        op0=Alu.mult, op1=Alu.mult,
                    )

                    # ---- normalize + store
                    for i, mt in enumerate(mts):
                        ot = out_p.tile([128, H], F32, tag="outp")
                        nc.scalar.activation(
                            ot,
                            t3s[i],
                            Act.Identity,
                            bias=nmr[:, i : i + 1],
                            scale=rstd[:, i : i + 1],
                        )
                        if use_gbe[s]:
                            gb, beb = gbe_rows[s]
                            nc.vector.tensor_tensor(
                                out=ot, in0=ot, in1=gb, op=Alu.mult
                            )
                            nc.vector.tensor_tensor(
                                out=ot, in0=ot, in1=beb, op=Alu.add
                            )
                        nc.sync.dma_start(
                            out=outs[s][mt * 128 : (mt + 1) * 128, :], in_=ot
                        )

    _split_sync_waits(nc)
    return nc


def _get_program(use_b, use_gbe):
    key = (use_b, use_gbe)
    if key not in _PROG_CACHE:
        _PROG_CACHE[key] = _build_program(use_b, use_gbe)
    return _PROG_CACHE[key]


# ------------------------------------------------------------------ kernel
def kernel(hs1, hs2, input_tensor, W1, b1, W2, b2, g1, be1, g2, be2,
           _trace=False):
    hs1 = np.asarray(hs1, dtype=np.float32)
    hs2 = np.asarray(hs2, dtype=np.float32)
    input_tensor = np.asarray(input_tensor, dtype=np.float32)
    W1 = np.asarray(W1, dtype=np.float32)
    W2 = np.asarray(W2, dtype=np.float32)
    b1 = np.asarray(b1, dtype=np.float32)
    b2 = np.asarray(b2, dtype=np.float32)
    g1 = np.asarray(g1, dtype=np.float32)
    g2 = np.asarray(g2, dtype=np.float32)
    be1 = np.asarray(be1, dtype=np.float32)
    be2 = np.asarray(be2, dtype=np.float32)

    use_b = (bool(np.any(b1 != 0.0)), bool(np.any(b2 != 0.0)))
    use_gbe = (
        bool(np.any(g1 != 1.0) or np.any(be1 != 0.0)),
        bool(np.any(g2 != 1.0) or np.any(be2 != 0.0)),
    )
    nc = _get_program(use_b, use_gbe)

    m1, m2 = _dropout_masks()
    w1t = np.ascontiguousarray(W1.T)
    w2t = np.ascontiguousarray(W2.T)
    hs1r = hs1.reshape(N_CORES, M, H)
    hs2r = hs2.reshape(N_CORES, M, H)
    inpr = input_tensor.reshape(N_CORES, M, H)

    in_maps = []
    for c in range(N_CORES):
        im = {
            "hs1t": np.ascontiguousarray(hs1r[c].T),
            "hs2t": np.ascontiguousarray(hs2r[c].T),
            "inp": np.ascontiguousarray(inpr[c]),
            "m1": m1[c],
            "m2": m2[c],
            "w1t": w1t,
            "w2t": w2t,
        }
        for s, (b, g, be) in enumerate(((b1, g1, be1), (b2, g2, be2))):
            if use_b[s]:
                im[f"bs{s}"] = np.ascontiguousarray(
                    (b.astype(np.float64) / (SCALE * SCALE)).astype(np.float32)
                )[None, :]
            if use_gbe[s]:
                im[f"g{s}"] = np.ascontiguousarray(g)[None, :]
                im[f"be{s}"] = np.ascontiguousarray(be)[None, :]
        in_maps.append(im)

    res = run_bass_kernel_spmd(
        nc, in_maps, core_ids=list(range(N_CORES)), trace=_trace
    )
    out1 = np.concatenate(
        [res.results[c]["out1"][None] for c in range(N_CORES)], axis=0
    ).reshape(B, S, H)
    out2 = np.concatenate(
        [res.results[c]["out2"][None] for c in range(N_CORES)], axis=0
    ).reshape(B, S, H)
    if _trace:
        return (out1, out2), res
    return (out1, out2)


# revision 6
# speedup vs baseline: 2.2914x; 2.2914x over previous
"""Trainium2 Bass kernel for nn_BertSelfOutput_79448305042105.

Computes, for two streams (hs1,W1,b1,g1,be1) and (hs2,W2,b2,g2,be2):
    h   = quant(hs) @ quant(W).T + b          (symmetric 8-bit quant-dequant)
    h   = dropout(h, p=0.1, jax key 42)
    out = layernorm(h + input_tensor) * g + be

Sharding: data-parallel over batch, 2 batches (2048 tokens) per core, 8 cores.

Numerics: quantized values are integers in [-127, 127], exactly representable
in bf16; integer products accumulate exactly in fp32 PSUM (max |sum| < 2^24),
so the matmul runs at full bf16 speed with zero quantization-grid error.
f32->int32 conversion on the vector/scalar engines is round-to-nearest-even,
matching jnp.round. Dropout masks depend only on the fixed PRNG key, so they
are precomputed on host (uint8) and applied on-device.
"""

import sys

if "/opt/trn_rl_repo" not in sys.path:
    sys.path.insert(0, "/opt/trn_rl_repo")

import numpy as np

import concourse.bass as bass
import concourse.tile as tile
from concourse import mybir
from concourse.bass_utils import run_bass_kernel_spmd

# ---------------------------------------------------------------- constants
B, S, H = 16, 1024, 1024
N_CORES = 8
B_PER_CORE = B // N_CORES            # 2
M = B_PER_CORE * S                   # 2048 tokens per core
M_TILES = M // 128                   # 16
H_TILES = H // 128                   # 8
O_CHUNK = 512                        # psum bank free dim
O_CHUNKS = H // O_CHUNK              # 2
GRP = 4                              # m-tiles per stats group
N_GRPS = M_TILES // GRP

CLIP = 2.5
WBITS = IBITS = 8
QN = float(2 ** (IBITS - 1) - 1)     # 127
SCALE = CLIP / QN                    # quant step
INV_SCALE = np.float32(QN / CLIP)
KEEP = 0.9
SP = np.float32((CLIP / QN) * (CLIP / QN) / KEEP)   # psum*SP = dequant/keep
EPS = 1e-12

F32 = mybir.dt.float32
BF16 = mybir.dt.bfloat16
I32 = mybir.dt.int32
U8 = mybir.dt.uint8
Alu = mybir.AluOpType
Act = mybir.ActivationFunctionType

_MASK_CACHE = {}
_PROG_CACHE = {}


# ------------------------------------------------------- walrus wait limit
def _split_sync_waits(nc, max_waits=1):
    """This toolchain's walrus rejects instructions with >1 sync wait.
    Move extra waits onto same-engine NoOps placed just before."""
    ctr = 0
    for f in nc.m.functions:
        for bb in f.blocks:
            out, changed = [], False
            for ins in bb.instructions:
                si = getattr(ins, "sync_info", None)
                waits = list(si.on_wait) if (si is not None and si.on_wait) else []
                if len(waits) > max_waits:
                    changed = True
                    for w in waits[:-max_waits]:
                        ctr += 1
                        out.append(
                            mybir.InstNoOp(
                                name=f"WSPLIT-{ctr}",
                                sync_info=mybir.SyncInfo(on_wait=[w], on_update=[]),
                                bass_nofuse=True,
                                engine=ins.engine,
                            )
                        )
                    ins.sync_info = mybir.SyncInfo(
                        on_wait=waits[-max_waits:],
                        on_update=list(si.on_update) if si.on_update else [],
                    )
                out.append(ins)
            if changed:
                bb.instructions = out


# ------------------------------------------------------------ host helpers
def _dropout_masks():
    """uint8 keep-masks for the two dropout calls (jax key 42), host-cached."""
    if "m" not in _MASK_CACHE:
        import jax

        cpu = jax.devices("cpu")[0]
        with jax.default_device(cpu):
            dkey = jax.random.key(42)
            k1, k2 = jax.random.split(dkey)
            m1 = np.asarray(
                jax.random.bernoulli(k1, KEEP, (B, S, H))
            ).astype(np.uint8)
            m2 = np.asarray(
                jax.random.bernoulli(k2, KEEP, (B, S, H))
            ).astype(np.uint8)
        _MASK_CACHE["m"] = (
            np.ascontiguousarray(m1.reshape(N_CORES, M, H)),
            np.ascontiguousarray(m2.reshape(N_CORES, M, H)),
        )
    return _MASK_CACHE["m"]


# ----------------------------------------------------------- program build
def _build_program(use_b, use_gbe):
    """use_b/use_gbe: per-stream flags for nonzero bias / nontrivial g,be."""
    nc = bass.Bass()

    hs1t = nc.declare_dram_parameter("hs1t", [H, M], F32, isOutput=False)
    hs2t = nc.declare_dram_parameter("hs2t", [H, M], F32, isOutput=False)
    inp = nc.declare_dram_parameter("inp", [M, H], F32, isOutput=False)
    m1 = nc.declare_dram_parameter("m1", [M, H], U8, isOutput=False)
    m2 = nc.declare_dram_parameter("m2", [M, H], U8, isOutput=False)
    w1t = nc.declare_dram_parameter("w1t", [H, H], F32, isOutput=False)
    w2t = nc.declare_dram_parameter("w2t", [H, H], F32, isOutput=False)
    extras = {}
    for s in (0, 1):
        if use_b[s]:
            # pre-divided by SCALE^2 on host: accumulated into integer psum
            extras[f"bs{s}"] = nc.declare_dram_parameter(
                f"bs{s}", [1, H], F32, isOutput=False
            )
        if use_gbe[s]:
            extras[f"g{s}"] = nc.declare_dram_parameter(
                f"g{s}", [1, H], F32, isOutput=False
            )
            extras[f"be{s}"] = nc.declare_dram_parameter(
                f"be{s}", [1, H], F32, isOutput=False
            )
    out1 = nc.declare_dram_parameter("out1", [M, H], F32, isOutput=True)
    out2 = nc.declare_dram_parameter("out2", [M, H], F32, isOutput=True)

    hst = (hs1t, hs2t)
    msk = (m1, m2)
    wt = (w1t, w2t)
    outs = (out1, out2)

    with tile.TileContext(nc) as tc:
        import contextlib

        with contextlib.ExitStack() as ctx:
            const_p = ctx.enter_context(tc.tile_pool(name="const", bufs=1))
            wq_p = ctx.enter_context(tc.tile_pool(name="wq", bufs=1))
            xq_p = ctx.enter_context(tc.tile_pool(name="xq", bufs=1))
            stage_f = ctx.enter_context(tc.tile_pool(name="stage_f", bufs=2))
            stage_i = ctx.enter_context(tc.tile_pool(name="stage_i", bufs=2))
            inp_p = ctx.enter_context(tc.tile_pool(name="inp", bufs=5))
            msk_p = ctx.enter_context(tc.tile_pool(name="msk", bufs=4))
            t2_p = ctx.enter_context(tc.tile_pool(name="t2", bufs=3))
            t3_p = ctx.enter_context(tc.tile_pool(name="t3", bufs=8))
            out_p = ctx.enter_context(tc.tile_pool(name="outp", bufs=3))
            st_p = ctx.enter_context(tc.tile_pool(name="stats", bufs=4))
            psum_p = ctx.enter_context(
                tc.tile_pool(name="psum", bufs=8, space="PSUM")
            )

            eps_t = const_p.tile([128, 1], F32)
            nc.vector.memset(eps_t, EPS)

            # one-partition constants for optional bias / gamma / beta
            bias_rows = {}
            gbe_rows = {}
            ones_col = None
            for s in (0, 1):
                if use_b[s]:
                    bs = const_p.tile([1, H], F32, tag=f"bs{s}")
                    nc.sync.dma_start(out=bs, in_=extras[f"bs{s}"][:, :])
                    bias_rows[s] = bs
                    if ones_col is None:
                        ones_col = const_p.tile([1, 128], F32)
                        nc.vector.memset(ones_col, 1.0)
                if use_gbe[s]:
                    gb = const_p.tile([128, H], F32, tag=f"gb{s}")
                    beb = const_p.tile([128, H], F32, tag=f"beb{s}")
                    src_g = extras[f"g{s}"]
                    src_be = extras[f"be{s}"]
                    # partition-broadcast DMA: replicate the [1, H] row
                    nc.sync.dma_start(
                        out=gb,
                        in_=bass.AP(
                            tensor=src_g.tensor if hasattr(src_g, "tensor") else src_g,
                            offset=0,
                            ap=[[0, 128], [1, H]],
                        ),
                    )
                    nc.sync.dma_start(
                        out=beb,
                        in_=bass.AP(
                            tensor=src_be.tensor if hasattr(src_be, "tensor") else src_be,
                            offset=0,
                            ap=[[0, 128], [1, H]],
                        ),
                    )
                    gbe_rows[s] = (gb, beb)

            # ---- quantize weights + activations, stream-major so that
            # stream 0's matmuls can start while stream 1 still loads.
            # W: DVE round->i32, DVE clip->bf16 (prologue, DVE idle).
            # X: ACT round->i32, GPSIMD clip->bf16 (keeps DVE free for the
            #    steady-state psum/residual chains).
            wq, xq = [], []
            for s in (0, 1):
                wqs = wq_p.tile([128, H_TILES, H], BF16, tag=f"wq{s}")
                for h in range(H_TILES):
                    for col in range(2):
                        fs = stage_f.tile([128, O_CHUNK], F32, tag="stage_f")
                        nc.sync.dma_start(
                            out=fs,
                            in_=wt[s][
                                h * 128 : (h + 1) * 128,
                                col * O_CHUNK : (col + 1) * O_CHUNK,
                            ],
                        )
                        ii = stage_i.tile([128, O_CHUNK], I32, tag="stage_i")
                        nc.vector.tensor_scalar(
                            ii, fs, float(INV_SCALE), None, Alu.mult
                        )
                        nc.vector.tensor_scalar(
                            wqs[:, h, col * O_CHUNK : (col + 1) * O_CHUNK],
                            ii,
                            -127,
                            127,
                            Alu.max,
                            Alu.min,
                        )
                wq.append(wqs)
                xqs = xq_p.tile([128, H_TILES, M], BF16, tag=f"xq{s}")
                for h in range(H_TILES):
                    for col in range(2):
                        fs = stage_f.tile([128, M // 2], F32, tag="stage_f")
                        nc.sync.dma_start(
                            out=fs,
                            in_=hst[s][
                                h * 128 : (h + 1) * 128,
                                col * (M // 2) : (col + 1) * (M // 2),
                            ],
                        )
                        ii = stage_i.tile([128, M // 2], I32, tag="stage_i")
                        nc.scalar.activation(
                            ii, fs, Act.Copy, bias=0.0, scale=float(INV_SCALE)
                        )
                        nc.vector.tensor_scalar(
                            xqs[:, h, col * (M // 2) : (col + 1) * (M // 2)],
                            ii,
                            -127,
                            127,
                            Alu.max,
                            Alu.min,
                        )
                xq.append(xqs)

            # ---- main loop
            inv_h = 1.0 / float(H)
            for grp in range(N_GRPS):
                mts = [grp * GRP + i for i in range(GRP)]
                inp_tiles = []
                for mt in mts:
                    it = inp_p.tile([128, H], F32, tag="inp")
                    nc.sync.dma_start(
                        out=it, in_=inp[mt * 128 : (mt + 1) * 128, :]
                    )
                    inp_tiles.append(it)

                for s in (0, 1):
                    gsum = st_p.tile([128, GRP], F32, tag="gsum")
                    gssq = st_p.tile([128, GRP], F32, tag="gssq")
                    t3s = []
                    for i, mt in enumerate(mts):
                        mk = msk_p.tile([128, H], U8, tag="msk")
                        nc.sync.dma_start(
                            out=mk, in_=msk[s][mt * 128 : (mt + 1) * 128, :]
                        )
                        # matmul: psum[oc] = sum_h xq[:,h,mt]^T @ wq[:,h,oc]
                        # h-major: both o-chunks of one h share the
                        # stationary operand back-to-back.
                        pss = [
                            psum_p.tile(
                                [128, O_CHUNK], F32, tag="ps", name=f"ps{_oc}"
                            )
                            for _oc in range(O_CHUNKS)
                        ]
                        for h in range(H_TILES):
                            for oc in range(O_CHUNKS):
                                last = h == H_TILES - 1 and not use_b[s]
                                nc.tensor.matmul(
                                    pss[oc],
                                    xq[s][:, h, mt * 128 : (mt + 1) * 128],
                                    wq[s][:, h, oc * O_CHUNK : (oc + 1) * O_CHUNK],
                                    start=(h == 0),
                                    stop=last,
                                )
                        if use_b[s]:
                            for oc in range(O_CHUNKS):
                                nc.tensor.matmul(
                                    pss[oc],
                                    ones_col,
                                    bias_rows[s][
                                        :, oc * O_CHUNK : (oc + 1) * O_CHUNK
                                    ],
                                    start=False,
                                    stop=True,
                                )
                        # dropout+dequant: t2 = (psum * SP) * mask
                        t2 = t2_p.tile([128, H], F32, tag="t2")
                        for oc in range(O_CHUNKS):
                            nc.vector.scalar_tensor_tensor(
                                out=t2[:, oc * O_CHUNK : (oc + 1) * O_CHUNK],
                                in0=pss[oc],
                                scalar=float(SP),
                                in1=mk[:, oc * O_CHUNK : (oc + 1) * O_CHUNK],
                                op0=Alu.mult,
                                op1=Alu.mult,
                            )
                        # residual + row-sum
                        t3 = t3_p.tile([128, H], F32, tag="t3")
                        nc.vector.scalar_tensor_tensor(
                            out=t3,
                            in0=t2,
                            scalar=0.0,
                            in1=inp_tiles[i],
                            op0=Alu.add,
                            op1=Alu.add,
                            accum_out=gsum[:, i : i + 1],
                        )
                        t3s.append(t3)
                        # sum of squares on scalar engine (reuse dead t2)
                        nc.scalar.activation(
                            t2, t3, Act.Square, accum_out=gssq[:, i : i + 1]
                        )

                    # ---- group stats: mu, var, rstd (one Newton step)
                    mu = st_p.tile([128, GRP], F32, tag="mu")
                    nc.vector.tensor_scalar(mu, gsum, inv_h, None, Alu.mult)
                    musq = st_p.tile([128, GRP], F32, tag="musq")
                    nc.vector.tensor_tensor(
                        out=musq, in0=mu, in1=mu, op=Alu.mult
                    )
                    var = st_p.tile([128, GRP], F32, tag="var")
                    nc.vector.scalar_tensor_tensor(
                        out=var,
                        in0=gssq,
                        scalar=inv_h,
                        in1=musq,
                        op0=Alu.mult,
                        op1=Alu.subtract,
                    )
                    std = st_p.tile([128, GRP], F32, tag="std")
                    nc.scalar.activation(
                        std, var, Act.Sqrt, bias=eps_t, scale=1.0
                    )
                    r0 = st_p.tile([128, GRP], F32, tag="r0")
                    nc.vector.reciprocal(r0, std)
                    # Newton: rstd = r0 * (1.5 - 0.5 * vpe * r0^2)
                    vpe = st_p.tile([128, GRP], F32, tag="vpe")
                    nc.vector.tensor_scalar(vpe, var, EPS, None, Alu.add)
                    r2 = st_p.tile([128, GRP], F32, tag="r2")
                    nc.vector.tensor_tensor(out=r2, in0=r0, in1=r0, op=Alu.mult)
                    f = st_p.tile([128, GRP], F32, tag="f")
                    nc.vector.tensor_tensor(out=f, in0=r2, in1=vpe, op=Alu.mult)
                    g = st_p.tile([128, GRP], F32, tag="g")
                    nc.vector.tensor_scalar(g, f, -0.5, 1.5, Alu.mult, Alu.add)
                    rstd = st_p.tile([128, GRP], F32, tag="rstd")
                    nc.vector.tensor_tensor(out=rstd, in0=r0, in1=g, op=Alu.mult)
                    nmr = st_p.tile([128, GRP], F32, tag="nmr")
                    nc.vector.scalar_tensor_tensor(
                        out=nmr, in0=mu, scalar=-1.0, in1=rstd,
                        op0=Alu.mult, op1=Alu.mult,
                    )

                    # ---- normalize + store
                    for i, mt in enumerate(mts):
                        ot = out_p.tile([128, H], F32, tag="outp")
                        nc.scalar.activation(
                            ot,
                            t3s[i],
                            Act.Identity,
                            bias=nmr[:, i : i + 1],
                            scale=rstd[:, i : i + 1],
                        )
                        if use_gbe[s]:
                            gb, beb = gbe_rows[s]
                            nc.vector.tensor_tensor(
                                out=ot, in0=ot, in1=gb, op=Alu.mult
                            )
                            nc.vector.tensor_tensor(
                                out=ot, in0=ot, in1=beb, op=Alu.add
                            )
                        nc.sync.dma_start(
                            out=outs[s][mt * 128 : (mt + 1) * 128, :], in_=ot
                        )

    _split_sync_waits(nc)
    return nc


def _get_program(use_b, use_gbe):
    key = (use_b, use_gbe)
    if key not in _PROG_CACHE:
        _PROG_CACHE[key] = _build_program(use_b, use_gbe)
    return _PROG_CACHE[key]


# ------------------------------------------------------------------ kernel
def kernel(hs1, hs2, input_tensor, W1, b1, W2, b2, g1, be1, g2, be2,
           _trace=False):
    hs1 = np.asarray(hs1, dtype=np.float32)
    hs2 = np.asarray(hs2, dtype=np.float32)
    input_tensor = np.asarray(input_tensor, dtype=np.float32)
    W1 = np.asarray(W1, dtype=np.float32)
    W2 = np.asarray(W2, dtype=np.float32)
    b1 = np.asarray(b1, dtype=np.float32)
    b2 = np.asarray(b2, dtype=np.float32)
    g1 = np.asarray(g1, dtype=np.float32)
    g2 = np.asarray(g2, dtype=np.float32)
    be1 = np.asarray(be1, dtype=np.float32)
    be2 = np.asarray(be2, dtype=np.float32)

    use_b = (bool(np.any(b1 != 0.0)), bool(np.any(b2 != 0.0)))
    use_gbe = (
        bool(np.any(g1 != 1.0) or np.any(be1 != 0.0)),
        bool(np.any(g2 != 1.0) or np.any(be2 != 0.0)),
    )
    nc = _get_program(use_b, use_gbe)

    m1, m2 = _dropout_masks()
    w1t = np.ascontiguousarray(W1.T)
    w2t = np.ascontiguousarray(W2.T)
    hs1r = hs1.reshape(N_CORES, M, H)
    hs2r = hs2.reshape(N_CORES, M, H)
    inpr = input_tensor.reshape(N_CORES, M, H)

    in_maps = []
    for c in range(N_CORES):
        im = {
            "hs1t": np.ascontiguousarray(hs1r[c].T),
            "hs2t": np.ascontiguousarray(hs2r[c].T),
            "inp": np.ascontiguousarray(inpr[c]),
            "m1": m1[c],
            "m2": m2[c],
            "w1t": w1t,
            "w2t": w2t,
        }
        for s, (b, g, be) in enumerate(((b1, g1, be1), (b2, g2, be2))):
            if use_b[s]:
                im[f"bs{s}"] = np.ascontiguousarray(
                    (b.astype(np.float64) / (SCALE * SCALE)).astype(np.float32)
                )[None, :]
            if use_gbe[s]:
                im[f"g{s}"] = np.ascontiguousarray(g)[None, :]
                im[f"be{s}"] = np.ascontiguousarray(be)[None, :]
        in_maps.append(im)

    res = run_bass_kernel_spmd(
        nc, in_maps, core_ids=list(range(N_CORES)), trace=_trace
    )
    out1 = np.concatenate(
        [res.results[c]["out1"][None] for c in range(N_CORES)], axis=0
    ).reshape(B, S, H)
    out2 = np.concatenate(
        [res.results[c]["out2"][None] for c in range(N_CORES)], axis=0
    ).reshape(B, S, H)
    if _trace:
        return (out1, out2), res
    return (out1, out2)


# revision 7
# speedup vs baseline: 2.7603x; 1.2046x over previous
"""Trainium2 Bass kernel for nn_BertSelfOutput_79448305042105.

Computes, for two streams (hs1,W1,b1,g1,be1) and (hs2,W2,b2,g2,be2):
    h   = quant(hs) @ quant(W).T + b          (symmetric 8-bit quant-dequant)
    h   = dropout(h, p=0.1, jax key 42)
    out = layernorm(h + input_tensor) * g + be

Sharding: data-parallel over batch, 2 batches (2048 tokens) per core, 8 cores.

Numerics: quantized values are integers in [-127, 127], exactly representable
in bf16; integer products accumulate exactly in fp32 PSUM (max |sum| < 2^24),
so the matmul runs at full bf16 speed with zero quantization-grid error.
f32->int32 conversion on the vector/scalar engines is round-to-nearest-even,
matching jnp.round. Dropout masks depend only on the fixed PRNG key, so they
are precomputed on host (uint8) and applied on-device.
"""

import sys

if "/opt/trn_rl_repo" not in sys.path:
    sys.path.insert(0, "/opt/trn_rl_repo")

import numpy as np

import concourse.bass as bass
import concourse.tile as tile
from concourse import mybir
from concourse.bass_utils import run_bass_kernel_spmd

# ---------------------------------------------------------------- constants
B, S, H = 16, 1024, 1024
N_CORES = 8
B_PER_CORE = B // N_CORES            # 2
M = B_PER_CORE * S                   # 2048 tokens per core
M_TILES = M // 128                   # 16
H_TILES = H // 128                   # 8
O_CHUNK = 512                        # psum bank free dim
O_CHUNKS = H // O_CHUNK              # 2
GRP = 4                              # m-tiles per stats group
N_GRPS = M_TILES // GRP

CLIP = 2.5
WBITS = IBITS = 8
QN = float(2 ** (IBITS - 1) - 1)     # 127
SCALE = CLIP / QN                    # quant step
INV_SCALE = np.float32(QN / CLIP)
KEEP = 0.9
SP = np.float32((CLIP / QN) * (CLIP / QN) / KEEP)   # psum*SP = dequant/keep
EPS = 1e-12

F32 = mybir.dt.float32
BF16 = mybir.dt.bfloat16
I32 = mybir.dt.int32
U8 = mybir.dt.uint8
Alu = mybir.AluOpType
Act = mybir.ActivationFunctionType

_MASK_CACHE = {}
_PROG_CACHE = {}


# ------------------------------------------------------- walrus wait limit
def _split_sync_waits(nc, max_waits=1):
    """This toolchain's walrus rejects instructions with >1 sync wait.
    Move extra waits onto same-engine NoOps placed just before."""
    ctr = 0
    for f in nc.m.functions:
        for bb in f.blocks:
            out, changed = [], False
            for ins in bb.instructions:
                si = getattr(ins, "sync_info", None)
                waits = list(si.on_wait) if (si is not None and si.on_wait) else []
                if len(waits) > max_waits:
                    changed = True
                    for w in waits[:-max_waits]:
                        ctr += 1
                        out.append(
                            mybir.InstNoOp(
                                name=f"WSPLIT-{ctr}",
                                sync_info=mybir.SyncInfo(on_wait=[w], on_update=[]),
                                bass_nofuse=True,
                                engine=ins.engine,
                            )
                        )
                    ins.sync_info = mybir.SyncInfo(
                        on_wait=waits[-max_waits:],
                        on_update=list(si.on_update) if si.on_update else [],
                    )
                out.append(ins)
            if changed:
                bb.instructions = out


# ------------------------------------------------------------ host helpers
def _dropout_masks():
    """uint8 keep-masks for the two dropout calls (jax key 42), host-cached."""
    if "m" not in _MASK_CACHE:
        import jax

        cpu = jax.devices("cpu")[0]
        with jax.default_device(cpu):
            dkey = jax.random.key(42)
            k1, k2 = jax.random.split(dkey)
            m1 = np.asarray(
                jax.random.bernoulli(k1, KEEP, (B, S, H))
            ).astype(np.uint8)
            m2 = np.asarray(
                jax.random.bernoulli(k2, KEEP, (B, S, H))
            ).astype(np.uint8)
        _MASK_CACHE["m"] = (
            np.ascontiguousarray(m1.reshape(N_CORES, M, H)),
            np.ascontiguousarray(m2.reshape(N_CORES, M, H)),
        )
    return _MASK_CACHE["m"]


# ----------------------------------------------------------- program build
def _build_program(use_b, use_gbe):
    """use_b/use_gbe: per-stream flags for nonzero bias / nontrivial g,be."""
    nc = bass.Bass()

    hs1t = nc.declare_dram_parameter("hs1t", [H, M], F32, isOutput=False)
    hs2t = nc.declare_dram_parameter("hs2t", [H, M], F32, isOutput=False)
    inp = nc.declare_dram_parameter("inp", [M, H], F32, isOutput=False)
    m1 = nc.declare_dram_parameter("m1", [M, H], U8, isOutput=False)
    m2 = nc.declare_dram_parameter("m2", [M, H], U8, isOutput=False)
    w1t = nc.declare_dram_parameter("w1t", [H, H], F32, isOutput=False)
    w2t = nc.declare_dram_parameter("w2t", [H, H], F32, isOutput=False)
    extras = {}
    for s in (0, 1):
        if use_b[s]:
            # pre-divided by SCALE^2 on host: accumulated into integer psum
            extras[f"bs{s}"] = nc.declare_dram_parameter(
                f"bs{s}", [1, H], F32, isOutput=False
            )
        if use_gbe[s]:
            extras[f"g{s}"] = nc.declare_dram_parameter(
                f"g{s}", [1, H], F32, isOutput=False
            )
            extras[f"be{s}"] = nc.declare_dram_parameter(
                f"be{s}", [1, H], F32, isOutput=False
            )
    out1 = nc.declare_dram_parameter("out1", [M, H], F32, isOutput=True)
    out2 = nc.declare_dram_parameter("out2", [M, H], F32, isOutput=True)

    hst = (hs1t, hs2t)
    msk = (m1, m2)
    wt = (w1t, w2t)
    outs = (out1, out2)

    with tile.TileContext(nc) as tc:
        import contextlib

        with contextlib.ExitStack() as ctx:
            const_p = ctx.enter_context(tc.tile_pool(name="const", bufs=1))
            wq_p = ctx.enter_context(tc.tile_pool(name="wq", bufs=1))
            xq_p = ctx.enter_context(tc.tile_pool(name="xq", bufs=1))
            stage_f = ctx.enter_context(tc.tile_pool(name="stage_f", bufs=4))
            stage_i = ctx.enter_context(tc.tile_pool(name="stage_i", bufs=3))
            inp_p = ctx.enter_context(tc.tile_pool(name="inp", bufs=4))
            msk_p = ctx.enter_context(tc.tile_pool(name="msk", bufs=3))
            t2_p = ctx.enter_context(tc.tile_pool(name="t2", bufs=3))
            t3_p = ctx.enter_context(tc.tile_pool(name="t3", bufs=6))
            out_p = ctx.enter_context(tc.tile_pool(name="outp", bufs=3))
            st_p = ctx.enter_context(tc.tile_pool(name="stats", bufs=4))
            psum_p = ctx.enter_context(
                tc.tile_pool(name="psum", bufs=8, space="PSUM")
            )

            eps_t = const_p.tile([128, 1], F32)
            nc.vector.memset(eps_t, EPS)

            # one-partition constants for optional bias / gamma / beta
            bias_rows = {}
            gbe_rows = {}
            ones_col = None
            for s in (0, 1):
                if use_b[s]:
                    bs = const_p.tile([1, H], F32, tag=f"bs{s}")
                    nc.sync.dma_start(out=bs, in_=extras[f"bs{s}"][:, :])
                    bias_rows[s] = bs
                    if ones_col is None:
                        ones_col = const_p.tile([1, 128], F32)
                        nc.vector.memset(ones_col, 1.0)
                if use_gbe[s]:
                    gb = const_p.tile([128, H], F32, tag=f"gb{s}")
                    beb = const_p.tile([128, H], F32, tag=f"beb{s}")
                    src_g = extras[f"g{s}"]
                    src_be = extras[f"be{s}"]
                    # partition-broadcast DMA: replicate the [1, H] row
                    nc.sync.dma_start(
                        out=gb,
                        in_=bass.AP(
                            tensor=src_g.tensor if hasattr(src_g, "tensor") else src_g,
                            offset=0,
                            ap=[[0, 128], [1, H]],
                        ),
                    )
                    nc.sync.dma_start(
                        out=beb,
                        in_=bass.AP(
                            tensor=src_be.tensor if hasattr(src_be, "tensor") else src_be,
                            offset=0,
                            ap=[[0, 128], [1, H]],
                        ),
                    )
                    gbe_rows[s] = (gb, beb)

            # ---- per-stream: quantize W and X, then run the whole
            # stream's main loop before starting the next stream. This
            # keeps the PE busy with stream-s matmuls while stream s+1
            # loads/quantizes in the background.
            # W: DVE round->i32 + DVE clip->bf16 (prologue, DVE idle).
            # X: ACT round->i32 + DVE clip->bf16.
            wq, xq = [None, None], [None, None]

            def emit_quant(s):
                wqs = wq_p.tile([128, H_TILES, H], BF16, tag=f"wq{s}", name=f"wq{s}")
                for h in range(H_TILES):
                    fs = stage_f.tile([128, H], F32, tag="stage_f", name="wstage")
                    nc.sync.dma_start(out=fs, in_=wt[s][h * 128 : (h + 1) * 128, :])
                    ii = stage_i.tile([128, H], I32, tag="stage_i", name="wstagei")
                    nc.vector.tensor_scalar(
                        ii, fs, float(INV_SCALE), None, Alu.mult
                    )
                    nc.vector.tensor_scalar(
                        wqs[:, h, :], ii, -127, 127, Alu.max, Alu.min
                    )
                wq[s] = wqs
                xqs = xq_p.tile([128, H_TILES, M], BF16, tag=f"xq{s}", name=f"xq{s}")
                for h in range(H_TILES):
                    for col in range(2):
                        fs = stage_f.tile(
                            [128, M // 2], F32, tag="stage_f", name="xstage"
                        )
                        nc.sync.dma_start(
                            out=fs,
                            in_=hst[s][
                                h * 128 : (h + 1) * 128,
                                col * (M // 2) : (col + 1) * (M // 2),
                            ],
                        )
                        ii = stage_i.tile(
                            [128, M // 2], I32, tag="stage_i", name="xstagei"
                        )
                        nc.scalar.activation(
                            ii, fs, Act.Copy, bias=0.0, scale=float(INV_SCALE)
                        )
                        nc.vector.tensor_scalar(
                            xqs[:, h, col * (M // 2) : (col + 1) * (M // 2)],
                            ii,
                            -127,
                            127,
                            Alu.max,
                            Alu.min,
                        )
                xq[s] = xqs

            inv_h = 1.0 / float(H)

            def emit_stream(s):
                for grp in range(N_GRPS):
                    mts = [grp * GRP + i for i in range(GRP)]
                    inp_tiles = []
                    for mt in mts:
                        it = inp_p.tile([128, H], F32, tag="inp", name="inp")
                        nc.sync.dma_start(
                            out=it, in_=inp[mt * 128 : (mt + 1) * 128, :]
                        )
                        inp_tiles.append(it)

                    gsum = st_p.tile([128, GRP], F32, tag="gsum", name="gsum")
                    gssq = st_p.tile([128, GRP], F32, tag="gssq", name="gssq")
                    t3s = []
                    for i, mt in enumerate(mts):
                        mk = msk_p.tile([128, H], U8, tag="msk", name="msk")
                        nc.sync.dma_start(
                            out=mk, in_=msk[s][mt * 128 : (mt + 1) * 128, :]
                        )
                        # matmul: psum[oc] = sum_h xq[:,h,mt]^T @ wq[:,h,oc]
                        pss = [
                            psum_p.tile(
                                [128, O_CHUNK], F32, tag="ps", name=f"ps{_oc}"
                            )
                            for _oc in range(O_CHUNKS)
                        ]
                        for h in range(H_TILES):
                            for oc in range(O_CHUNKS):
                                last = h == H_TILES - 1 and not use_b[s]
                                nc.tensor.matmul(
                                    pss[oc],
                                    xq[s][:, h, mt * 128 : (mt + 1) * 128],
                                    wq[s][:, h, oc * O_CHUNK : (oc + 1) * O_CHUNK],
                                    start=(h == 0),
                                    stop=last,
                                )
                        if use_b[s]:
                            for oc in range(O_CHUNKS):
                                nc.tensor.matmul(
                                    pss[oc],
                                    ones_col,
                                    bias_rows[s][
                                        :, oc * O_CHUNK : (oc + 1) * O_CHUNK
                                    ],
                                    start=False,
                                    stop=True,
                                )
                        # dropout+dequant: t2 = (psum * SP) * mask
                        t2 = t2_p.tile([128, H], F32, tag="t2", name="t2")
                        for oc in range(O_CHUNKS):
                            nc.vector.scalar_tensor_tensor(
                                out=t2[:, oc * O_CHUNK : (oc + 1) * O_CHUNK],
                                in0=pss[oc],
                                scalar=float(SP),
                                in1=mk[:, oc * O_CHUNK : (oc + 1) * O_CHUNK],
                                op0=Alu.mult,
                                op1=Alu.mult,
                            )
                        # residual + row-sum
                        t3 = t3_p.tile([128, H], F32, tag="t3", name="t3")
                        nc.vector.scalar_tensor_tensor(
                            out=t3,
                            in0=t2,
                            scalar=0.0,
                            in1=inp_tiles[i],
                            op0=Alu.add,
                            op1=Alu.add,
                            accum_out=gsum[:, i : i + 1],
                        )
                        t3s.append(t3)
                        # sum of squares on scalar engine (reuse dead t2)
                        nc.scalar.activation(
                            t2, t3, Act.Square, accum_out=gssq[:, i : i + 1]
                        )

                    # ---- group stats: mu, var, rstd (one Newton step)
                    mu = st_p.tile([128, GRP], F32, tag="mu", name="mu")
                    nc.vector.tensor_scalar(mu, gsum, inv_h, None, Alu.mult)
                    musq = st_p.tile([128, GRP], F32, tag="musq", name="musq")
                    nc.vector.tensor_tensor(out=musq, in0=mu, in1=mu, op=Alu.mult)
                    var = st_p.tile([128, GRP], F32, tag="var", name="var")
                    nc.vector.scalar_tensor_tensor(
                        out=var,
                        in0=gssq,
                        scalar=inv_h,
                        in1=musq,
                        op0=Alu.mult,
                        op1=Alu.subtract,
                    )
                    std = st_p.tile([128, GRP], F32, tag="std", name="std")
                    nc.scalar.activation(std, var, Act.Sqrt, bias=eps_t, scale=1.0)
                    r0 = st_p.tile([128, GRP], F32, tag="r0", name="r0")
                    nc.vector.reciprocal(r0, std)
                    vpe = st_p.tile([128, GRP], F32, tag="vpe", name="vpe")
                    nc.vector.tensor_scalar(vpe, var, EPS, None, Alu.add)
                    r2 = st_p.tile([128, GRP], F32, tag="r2", name="r2")
                    nc.vector.tensor_tensor(out=r2, in0=r0, in1=r0, op=Alu.mult)
                    f = st_p.tile([128, GRP], F32, tag="f", name="f")
                    nc.vector.tensor_tensor(out=f, in0=r2, in1=vpe, op=Alu.mult)
                    g = st_p.tile([128, GRP], F32, tag="g", name="g")
                    nc.vector.tensor_scalar(g, f, -0.5, 1.5, Alu.mult, Alu.add)
                    rstd = st_p.tile([128, GRP], F32, tag="rstd", name="rstd")
                    nc.vector.tensor_tensor(out=rstd, in0=r0, in1=g, op=Alu.mult)
                    nmr = st_p.tile([128, GRP], F32, tag="nmr", name="nmr")
                    nc.vector.scalar_tensor_tensor(
                        out=nmr, in0=mu, scalar=-1.0, in1=rstd,
                        op0=Alu.mult, op1=Alu.mult,
                    )

                    # ---- normalize + store
                    for i, mt in enumerate(mts):
                        ot = out_p.tile([128, H], F32, tag="outp", name="outp")
                        nc.scalar.activation(
                            ot,
                            t3s[i],
                            Act.Identity,
                            bias=nmr[:, i : i + 1],
                            scale=rstd[:, i : i + 1],
                        )
                        if use_gbe[s]:
                            gb, beb = gbe_rows[s]
                            nc.vector.tensor_tensor(
                                out=ot, in0=ot, in1=gb, op=Alu.mult
                            )
                            nc.vector.tensor_tensor(
                                out=ot, in0=ot, in1=beb, op=Alu.add
                            )
                        nc.sync.dma_start(
                            out=outs[s][mt * 128 : (mt + 1) * 128, :], in_=ot
                        )

            emit_quant(0)
            emit_stream(0)
            emit_quant(1)
            emit_stream(1)

    _split_sync_waits(nc)
    return nc


def _get_program(use_b, use_gbe):
    key = (use_b, use_gbe)
    if key not in _PROG_CACHE:
        _PROG_CACHE[key] = _build_program(use_b, use_gbe)
    return _PROG_CACHE[key]


# ------------------------------------------------------------------ kernel
def kernel(hs1, hs2, input_tensor, W1, b1, W2, b2, g1, be1, g2, be2,
           _trace=False):
    hs1 = np.asarray(hs1, dtype=np.float32)
    hs2 = np.asarray(hs2, dtype=np.float32)
    input_tensor = np.asarray(input_tensor, dtype=np.float32)
    W1 = np.asarray(W1, dtype=np.float32)
    W2 = np.asarray(W2, dtype=np.float32)
    b1 = np.asarray(b1, dtype=np.float32)
    b2 = np.asarray(b2, dtype=np.float32)
    g1 = np.asarray(g1, dtype=np.float32)
    g2 = np.asarray(g2, dtype=np.float32)
    be1 = np.asarray(be1, dtype=np.float32)
    be2 = np.asarray(be2, dtype=np.float32)

    use_b = (bool(np.any(b1 != 0.0)), bool(np.any(b2 != 0.0)))
    use_gbe = (
        bool(np.any(g1 != 1.0) or np.any(be1 != 0.0)),
        bool(np.any(g2 != 1.0) or np.any(be2 != 0.0)),
    )
    nc = _get_program(use_b, use_gbe)

    m1, m2 = _dropout_masks()
    w1t = np.ascontiguousarray(W1.T)
    w2t = np.ascontiguousarray(W2.T)
    hs1r = hs1.reshape(N_CORES, M, H)
    hs2r = hs2.reshape(N_CORES, M, H)
    inpr = input_tensor.reshape(N_CORES, M, H)

    in_maps = []
    for c in range(N_CORES):
        im = {
            "hs1t": np.ascontiguousarray(hs1r[c].T),
            "hs2t": np.ascontiguousarray(hs2r[c].T),
            "inp": np.ascontiguousarray(inpr[c]),
            "m1": m1[c],
            "m2": m2[c],
            "w1t": w1t,
            "w2t": w2t,
        }
        for s, (b, g, be) in enumerate(((b1, g1, be1), (b2, g2, be2))):
            if use_b[s]:
                im[f"bs{s}"] = np.ascontiguousarray(
                    (b.astype(np.float64) / (SCALE * SCALE)).astype(np.float32)
                )[None, :]
            if use_gbe[s]:
                im[f"g{s}"] = np.ascontiguousarray(g)[None, :]
                im[f"be{s}"] = np.ascontiguousarray(be)[None, :]
        in_maps.append(im)

    res = run_bass_kernel_spmd(
        nc, in_maps, core_ids=list(range(N_CORES)), trace=_trace
    )
    out1 = np.concatenate(
        [res.results[c]["out1"][None] for c in range(N_CORES)], axis=0
    ).reshape(B, S, H)
    out2 = np.concatenate(
        [res.results[c]["out2"][None] for c in range(N_CORES)], axis=0
    ).reshape(B, S, H)
    if _trace:
        return (out1, out2), res
    return (out1, out2)


# revision 8
# speedup vs baseline: 3.0758x; 1.1143x over previous
"""Trainium2 Bass kernel for nn_BertSelfOutput_79448305042105.

Computes, for two streams (hs1,W1,b1,g1,be1) and (hs2,W2,b2,g2,be2):
    h   = quant(hs) @ quant(W).T + b          (symmetric 8-bit quant-dequant)
    h   = dropout(h, p=0.1, jax key 42)
    out = layernorm(h + input_tensor) * g + be

Sharding: data-parallel over batch, 2 batches (2048 tokens) per core, 8 cores.

Numerics: quantized values are integers in [-127, 127], exactly representable
in bf16; integer products accumulate exactly in fp32 PSUM (max |sum| < 2^24),
so the matmul runs at full bf16 speed with zero quantization-grid error.
f32->int32 conversion on the vector/scalar engines is round-to-nearest-even,
matching jnp.round. Dropout masks depend only on the fixed PRNG key, so they
are precomputed on host (uint8) and applied on-device.
"""

import sys

if "/opt/trn_rl_repo" not in sys.path:
    sys.path.insert(0, "/opt/trn_rl_repo")

import numpy as np

import concourse.bass as bass
import concourse.tile as tile
from concourse import mybir
from concourse.bass_utils import run_bass_kernel_spmd

# ---------------------------------------------------------------- constants
B, S, H = 16, 1024, 1024
N_CORES = 8
B_PER_CORE = B // N_CORES            # 2
M = B_PER_CORE * S                   # 2048 tokens per core
M_TILES = M // 128                   # 16
H_TILES = H // 128                   # 8
O_CHUNK = 512                        # psum bank free dim
O_CHUNKS = H // O_CHUNK              # 2
GRP = 4                              # m-tiles per stats group
N_GRPS = M_TILES // GRP

CLIP = 2.5
WBITS = IBITS = 8
QN = float(2 ** (IBITS - 1) - 1)     # 127
SCALE = CLIP / QN                    # quant step
INV_SCALE = np.float32(QN / CLIP)
KEEP = 0.9
SP = np.float32((CLIP / QN) * (CLIP / QN) / KEEP)   # psum*SP = dequant/keep
EPS = 1e-12

F32 = mybir.dt.float32
BF16 = mybir.dt.bfloat16
I32 = mybir.dt.int32
U8 = mybir.dt.uint8
Alu = mybir.AluOpType
Act = mybir.ActivationFunctionType

_MASK_CACHE = {}
_PROG_CACHE = {}


# ------------------------------------------------------- walrus wait limit
def _split_sync_waits(nc, max_waits=1):
    """This toolchain's walrus rejects instructions with >1 sync wait.
    Move extra waits onto same-engine NoOps placed just before."""
    ctr = 0
    for f in nc.m.functions:
        for bb in f.blocks:
            out, changed = [], False
            for ins in bb.instructions:
                si = getattr(ins, "sync_info", None)
                waits = list(si.on_wait) if (si is not None and si.on_wait) else []
                if len(waits) > max_waits:
                    changed = True
                    for w in waits[:-max_waits]:
                        ctr += 1
                        out.append(
                            mybir.InstNoOp(
                                name=f"WSPLIT-{ctr}",
                                sync_info=mybir.SyncInfo(on_wait=[w], on_update=[]),
                                bass_nofuse=True,
                                engine=ins.engine,
                            )
                        )
                    ins.sync_info = mybir.SyncInfo(
                        on_wait=waits[-max_waits:],
                        on_update=list(si.on_update) if si.on_update else [],
                    )
                out.append(ins)
            if changed:
                bb.instructions = out


# ------------------------------------------------------------ host helpers
def _dropout_masks():
    """uint8 keep-masks for the two dropout calls (jax key 42), host-cached."""
    if "m" not in _MASK_CACHE:
        import jax

        cpu = jax.devices("cpu")[0]
        with jax.default_device(cpu):
            dkey = jax.random.key(42)
            k1, k2 = jax.random.split(dkey)
            m1 = np.asarray(
                jax.random.bernoulli(k1, KEEP, (B, S, H))
            ).astype(np.uint8)
            m2 = np.asarray(
                jax.random.bernoulli(k2, KEEP, (B, S, H))
            ).astype(np.uint8)
        _MASK_CACHE["m"] = (
            np.ascontiguousarray(m1.reshape(N_CORES, M, H)),
            np.ascontiguousarray(m2.reshape(N_CORES, M, H)),
        )
    return _MASK_CACHE["m"]


# ----------------------------------------------------------- program build
def _build_program(use_b, use_gbe):
    """use_b/use_gbe: per-stream flags for nonzero bias / nontrivial g,be."""
    nc = bass.Bass()

    hs1t = nc.declare_dram_parameter("hs1t", [H, M], F32, isOutput=False)
    hs2t = nc.declare_dram_parameter("hs2t", [H, M], F32, isOutput=False)
    inp = nc.declare_dram_parameter("inp", [M, H], F32, isOutput=False)
    m1 = nc.declare_dram_parameter("m1", [M, H], U8, isOutput=False)
    m2 = nc.declare_dram_parameter("m2", [M, H], U8, isOutput=False)
    w1q8 = nc.declare_dram_parameter("w1q8", [H, H], mybir.dt.int8, isOutput=False)
    w2q8 = nc.declare_dram_parameter("w2q8", [H, H], mybir.dt.int8, isOutput=False)
    extras = {}
    for s in (0, 1):
        if use_b[s]:
            # pre-divided by SCALE^2 on host: accumulated into integer psum
            extras[f"bs{s}"] = nc.declare_dram_parameter(
                f"bs{s}", [1, H], F32, isOutput=False
            )
        if use_gbe[s]:
            extras[f"g{s}"] = nc.declare_dram_parameter(
                f"g{s}", [1, H], F32, isOutput=False
            )
            extras[f"be{s}"] = nc.declare_dram_parameter(
                f"be{s}", [1, H], F32, isOutput=False
            )
    out1 = nc.declare_dram_parameter("out1", [M, H], F32, isOutput=True)
    out2 = nc.declare_dram_parameter("out2", [M, H], F32, isOutput=True)

    hst = (hs1t, hs2t)
    msk = (m1, m2)
    wt = (w1q8, w2q8)
    outs = (out1, out2)

    with tile.TileContext(nc) as tc:
        import contextlib

        with contextlib.ExitStack() as ctx:
            const_p = ctx.enter_context(tc.tile_pool(name="const", bufs=1))
            wq_p = ctx.enter_context(tc.tile_pool(name="wq", bufs=1))
            xq_p = ctx.enter_context(tc.tile_pool(name="xq", bufs=1))
            stage_f = ctx.enter_context(tc.tile_pool(name="stage_f", bufs=3))
            stage_i = ctx.enter_context(tc.tile_pool(name="stage_i", bufs=2))
            inp_p = ctx.enter_context(tc.tile_pool(name="inp", bufs=10))
            msk_p = ctx.enter_context(tc.tile_pool(name="msk", bufs=2))
            t2_p = ctx.enter_context(tc.tile_pool(name="t2", bufs=2))
            t3_p = ctx.enter_context(tc.tile_pool(name="t3", bufs=5))
            out_p = ctx.enter_context(tc.tile_pool(name="outp", bufs=3))
            st_p = ctx.enter_context(tc.tile_pool(name="stats", bufs=4))
            w8_p = ctx.enter_context(tc.tile_pool(name="w8", bufs=2))
            psum_p = ctx.enter_context(
                tc.tile_pool(name="psum", bufs=8, space="PSUM")
            )

            eps_t = const_p.tile([128, 1], F32)
            nc.vector.memset(eps_t, EPS)

            # one-partition constants for optional bias / gamma / beta
            bias_rows = {}
            gbe_rows = {}
            ones_col = None
            for s in (0, 1):
                if use_b[s]:
                    bs = const_p.tile([1, H], F32, tag=f"bs{s}")
                    nc.sync.dma_start(out=bs, in_=extras[f"bs{s}"][:, :])
                    bias_rows[s] = bs
                    if ones_col is None:
                        ones_col = const_p.tile([1, 128], F32)
                        nc.vector.memset(ones_col, 1.0)
                if use_gbe[s]:
                    gb = const_p.tile([128, H], F32, tag=f"gb{s}")
                    beb = const_p.tile([128, H], F32, tag=f"beb{s}")
                    src_g = extras[f"g{s}"]
                    src_be = extras[f"be{s}"]
                    # partition-broadcast DMA: replicate the [1, H] row
                    nc.sync.dma_start(
                        out=gb,
                        in_=bass.AP(
                            tensor=src_g.tensor if hasattr(src_g, "tensor") else src_g,
                            offset=0,
                            ap=[[0, 128], [1, H]],
                        ),
                    )
                    nc.sync.dma_start(
                        out=beb,
                        in_=bass.AP(
                            tensor=src_be.tensor if hasattr(src_be, "tensor") else src_be,
                            offset=0,
                            ap=[[0, 128], [1, H]],
                        ),
                    )
                    gbe_rows[s] = (gb, beb)

            # ---- per-stream: quantize W and X, then run the whole
            # stream's main loop before starting the next stream. This
            # keeps the PE busy with stream-s matmuls while stream s+1
            # loads/quantizes in the background.
            # W: DVE round->i32 + DVE clip->bf16 (prologue, DVE idle).
            # X: ACT round->i32 + DVE clip->bf16.
            wq, xq = [None, None], [None, None]

            def emit_quant(s):
                wqs = wq_p.tile([128, H_TILES, H], BF16, tag=f"wq{s}", name=f"wq{s}")
                for h in range(H_TILES):
                    w8 = w8_p.tile([128, H], mybir.dt.int8, tag="w8", name="w8")
                    nc.sync.dma_start(out=w8, in_=wt[s][h * 128 : (h + 1) * 128, :])
                    nc.vector.tensor_copy(wqs[:, h, :], w8)
                wq[s] = wqs
                xqs = xq_p.tile([128, H_TILES, M], BF16, tag=f"xq{s}", name=f"xq{s}")
                for h in range(H_TILES):
                    for col in range(2):
                        fs = stage_f.tile(
                            [128, M // 2], F32, tag="stage_f", name="xstage"
                        )
                        nc.sync.dma_start(
                            out=fs,
                            in_=hst[s][
                                h * 128 : (h + 1) * 128,
                                col * (M // 2) : (col + 1) * (M // 2),
                            ],
                        )
                        ii = stage_i.tile(
                            [128, M // 2], I32, tag="stage_i", name="xstagei"
                        )
                        nc.scalar.activation(
                            ii, fs, Act.Copy, bias=0.0, scale=float(INV_SCALE)
                        )
                        nc.vector.tensor_scalar(
                            xqs[:, h, col * (M // 2) : (col + 1) * (M // 2)],
                            ii,
                            -127,
                            127,
                            Alu.max,
                            Alu.min,
                        )
                xq[s] = xqs

            inv_h = 1.0 / float(H)

            def emit_stream(s, phase_mts, inp_cache):
                for grp_mts in phase_mts:
                    mts = grp_mts
                    inp_tiles = []
                    for mt in mts:
                        if mt in inp_cache:
                            inp_tiles.append(inp_cache[mt])
                            continue
                        it = inp_p.tile([128, H], F32, tag="inp", name="inp")
                        nc.sync.dma_start(
                            out=it, in_=inp[mt * 128 : (mt + 1) * 128, :]
                        )
                        inp_cache[mt] = it
                        inp_tiles.append(it)

                    gsum = st_p.tile([128, GRP], F32, tag="gsum", name="gsum")
                    gssq = st_p.tile([128, GRP], F32, tag="gssq", name="gssq")
                    t3s = []
                    for i, mt in enumerate(mts):
                        mk = msk_p.tile([128, H], U8, tag="msk", name="msk")
                        nc.sync.dma_start(
                            out=mk, in_=msk[s][mt * 128 : (mt + 1) * 128, :]
                        )
                        # matmul: psum[oc] = sum_h xq[:,h,mt]^T @ wq[:,h,oc]
                        pss = [
                            psum_p.tile(
                                [128, O_CHUNK], F32, tag="ps", name=f"ps{_oc}"
                            )
                            for _oc in range(O_CHUNKS)
                        ]
                        for h in range(H_TILES):
                            for oc in range(O_CHUNKS):
                                last = h == H_TILES - 1 and not use_b[s]
                                nc.tensor.matmul(
                                    pss[oc],
                                    xq[s][:, h, mt * 128 : (mt + 1) * 128],
                                    wq[s][:, h, oc * O_CHUNK : (oc + 1) * O_CHUNK],
                                    start=(h == 0),
                                    stop=last,
                                )
                        if use_b[s]:
                            for oc in range(O_CHUNKS):
                                nc.tensor.matmul(
                                    pss[oc],
                                    ones_col,
                                    bias_rows[s][
                                        :, oc * O_CHUNK : (oc + 1) * O_CHUNK
                                    ],
                                    start=False,
                                    stop=True,
                                )
                        # dropout+dequant: t2 = (psum * SP) * mask
                        t2 = t2_p.tile([128, H], F32, tag="t2", name="t2")
                        for oc in range(O_CHUNKS):
                            nc.vector.scalar_tensor_tensor(
                                out=t2[:, oc * O_CHUNK : (oc + 1) * O_CHUNK],
                                in0=pss[oc],
                                scalar=float(SP),
                                in1=mk[:, oc * O_CHUNK : (oc + 1) * O_CHUNK],
                                op0=Alu.mult,
                                op1=Alu.mult,
                            )
                        # residual + row-sum
                        t3 = t3_p.tile([128, H], F32, tag="t3", name="t3")
                        nc.vector.scalar_tensor_tensor(
                            out=t3,
                            in0=t2,
                            scalar=0.0,
                            in1=inp_tiles[i],
                            op0=Alu.add,
                            op1=Alu.add,
                            accum_out=gsum[:, i : i + 1],
                        )
                        t3s.append(t3)
                        # sum of squares on scalar engine (reuse dead t2)
                        nc.scalar.activation(
                            t2, t3, Act.Square, accum_out=gssq[:, i : i + 1]
                        )

                    # ---- group stats: mu, var, rstd (one Newton step)
                    mu = st_p.tile([128, GRP], F32, tag="mu", name="mu")
                    nc.vector.tensor_scalar(mu, gsum, inv_h, None, Alu.mult)
                    musq = st_p.tile([128, GRP], F32, tag="musq", name="musq")
                    nc.vector.tensor_tensor(out=musq, in0=mu, in1=mu, op=Alu.mult)
                    var = st_p.tile([128, GRP], F32, tag="var", name="var")
                    nc.vector.scalar_tensor_tensor(
                        out=var,
                        in0=gssq,
                        scalar=inv_h,
                        in1=musq,
                        op0=Alu.mult,
                        op1=Alu.subtract,
                    )
                    std = st_p.tile([128, GRP], F32, tag="std", name="std")
                    nc.scalar.activation(std, var, Act.Sqrt, bias=eps_t, scale=1.0)
                    r0 = st_p.tile([128, GRP], F32, tag="r0", name="r0")
                    nc.vector.reciprocal(r0, std)
                    vpe = st_p.tile([128, GRP], F32, tag="vpe", name="vpe")
                    nc.vector.tensor_scalar(vpe, var, EPS, None, Alu.add)
                    r2 = st_p.tile([128, GRP], F32, tag="r2", name="r2")
                    nc.vector.tensor_tensor(out=r2, in0=r0, in1=r0, op=Alu.mult)
                    f = st_p.tile([128, GRP], F32, tag="f", name="f")
                    nc.vector.tensor_tensor(out=f, in0=r2, in1=vpe, op=Alu.mult)
                    g = st_p.tile([128, GRP], F32, tag="g", name="g")
                    nc.vector.tensor_scalar(g, f, -0.5, 1.5, Alu.mult, Alu.add)
                    rstd = st_p.tile([128, GRP], F32, tag="rstd", name="rstd")
                    nc.vector.tensor_tensor(out=rstd, in0=r0, in1=g, op=Alu.mult)
                    nmr = st_p.tile([128, GRP], F32, tag="nmr", name="nmr")
                    nc.vector.scalar_tensor_tensor(
                        out=nmr, in0=mu, scalar=-1.0, in1=rstd,
                        op0=Alu.mult, op1=Alu.mult,
                    )

                    # ---- normalize + store
                    for i, mt in enumerate(mts):
                        ot = out_p.tile([128, H], F32, tag="outp", name="outp")
                        nc.scalar.activation(
                            ot,
                            t3s[i],
                            Act.Identity,
                            bias=nmr[:, i : i + 1],
                            scale=rstd[:, i : i + 1],
                        )
                        if use_gbe[s]:
                            gb, beb = gbe_rows[s]
                            nc.vector.tensor_tensor(
                                out=ot, in0=ot, in1=gb, op=Alu.mult
                            )
                            nc.vector.tensor_tensor(
                                out=ot, in0=ot, in1=beb, op=Alu.add
                            )
                        nc.sync.dma_start(
                            out=outs[s][mt * 128 : (mt + 1) * 128, :], in_=ot
                        )

            first_half = [[i * GRP + j for j in range(GRP)] for i in range(2)]
            second_half = [[8 + i * GRP + j for j in range(GRP)] for i in range(2)]
            emit_quant(0)
            inp_cache_a = {}
            emit_stream(0, first_half, inp_cache_a)
            emit_quant(1)
            emit_stream(1, first_half, inp_cache_a)
            inp_cache_b = {}
            emit_stream(0, second_half, inp_cache_b)
            emit_stream(1, second_half, inp_cache_b)

    _split_sync_waits(nc)
    return nc


def _get_program(use_b, use_gbe):
    key = (use_b, use_gbe)
    if key not in _PROG_CACHE:
        _PROG_CACHE[key] = _build_program(use_b, use_gbe)
    return _PROG_CACHE[key]


# ------------------------------------------------------------------ kernel
def kernel(hs1, hs2, input_tensor, W1, b1, W2, b2, g1, be1, g2, be2,
           _trace=False):
    hs1 = np.asarray(hs1, dtype=np.float32)
    hs2 = np.asarray(hs2, dtype=np.float32)
    input_tensor = np.asarray(input_tensor, dtype=np.float32)
    W1 = np.asarray(W1, dtype=np.float32)
    W2 = np.asarray(W2, dtype=np.float32)
    b1 = np.asarray(b1, dtype=np.float32)
    b2 = np.asarray(b2, dtype=np.float32)
    g1 = np.asarray(g1, dtype=np.float32)
    g2 = np.asarray(g2, dtype=np.float32)
    be1 = np.asarray(be1, dtype=np.float32)
    be2 = np.asarray(be2, dtype=np.float32)

    use_b = (bool(np.any(b1 != 0.0)), bool(np.any(b2 != 0.0)))
    use_gbe = (
        bool(np.any(g1 != 1.0) or np.any(be1 != 0.0)),
        bool(np.any(g2 != 1.0) or np.any(be2 != 0.0)),
    )
    nc = _get_program(use_b, use_gbe)

    m1, m2 = _dropout_masks()
    w1q8 = np.ascontiguousarray(
        np.clip(np.rint(W1.T.astype(np.float64) * float(INV_SCALE)), -127, 127)
    ).astype(np.int8)
    w2q8 = np.ascontiguousarray(
        np.clip(np.rint(W2.T.astype(np.float64) * float(INV_SCALE)), -127, 127)
    ).astype(np.int8)
    hs1r = hs1.reshape(N_CORES, M, H)
    hs2r = hs2.reshape(N_CORES, M, H)
    inpr = input_tensor.reshape(N_CORES, M, H)

    in_maps = []
    for c in range(N_CORES):
        im = {
            "hs1t": np.ascontiguousarray(hs1r[c].T),
            "hs2t": np.ascontiguousarray(hs2r[c].T),
            "inp": np.ascontiguousarray(inpr[c]),
            "m1": m1[c],
            "m2": m2[c],
            "w1q8": w1q8,
            "w2q8": w2q8,
        }
        for s, (b, g, be) in enumerate(((b1, g1, be1), (b2, g2, be2))):
            if use_b[s]:
                im[f"bs{s}"] = np.ascontiguousarray(
                    (b.astype(np.float64) / (SCALE * SCALE)).astype(np.float32)
                )[None, :]
            if use_gbe[s]:
                im[f"g{s}"] = np.ascontiguousarray(g)[None, :]
                im[f"be{s}"] = np.ascontiguousarray(be)[None, :]
        in_maps.append(im)

    res = run_bass_kernel_spmd(
        nc, in_maps, core_ids=list(range(N_CORES)), trace=_trace
    )
    out1 = np.concatenate(
        [res.results[c]["out1"][None] for c in range(N_CORES)], axis=0
    ).reshape(B, S, H)
    out2 = np.concatenate(
        [res.results[c]["out2"][None] for c in range(N_CORES)], axis=0
    ).reshape(B, S, H)
    if _trace:
        return (out1, out2), res
    return (out1, out2)
